# revision 18
# baseline (speedup 1.0000x reference)
"""Trainium2 Bass kernel for DistributedAFNO2Dv2 (2,768,256,256 AFNO block).

Sharding: 16 (batch, block) units across 8 cores = 2 units/core; the channel
mix is block-diagonal (96 ch/block) so there is no cross-core communication.

Per unit, three passes over DRAM:
  pass1 (per channel): 2-D forward DFT as matmuls.
      stage1 contracts the row dim m (x stationary, packed [cos|sin|nyq]
      mover) -> T1^T with the col dim n on partitions; stage2 contracts n
      (T1^T stationary, packed [Cw|Sw]-style movers) -> Z rows in "ht" order
      (ht<=127: h=ht; ht>=128: h=383-ht, so ht=255 is the h=128 Nyquist row).
      Z stored bf16 to zd[unit][c][ht][ri][w].
  pass2: per-mode complex 2-layer MLP over the 96 block channels
      (contract c). Biases are folded in via a 97th ones-row; relu on ACT;
      softshrink as relu(u-lam) + (min(u,-lam)+lam) split DVE/GPSIMD.
      Output bf16 to sd (same layout as zd).
  pass3 (per channel): inverse DFT. stage A contracts ht (S stationary,
      packed [Er|Ei] movers, rows permuted to match ht order) -> U^T with
      w on partitions; stage B contracts w -> spatial out + residual.

All matmul operands are bf16 (fp32 PSUM accumulation); the residual add uses
the exact fp32 x. Measured model error vs fp64 reference: ~2.6e-4.
"""

import numpy as np

H = 256
W = 256
WF = 129
CB = 96          # channels per block
NB = 8
LAM = 0.01
NCORES = 8
UNITS_PER_CORE = 2
CG = 4           # channels per DMA group in pass1/pass3
HTG = 16         # ht rows per pass2 tile

_cache = {}


def _make_consts():
    f = np.float64
    m = np.arange(H, dtype=f)[:, None]
    h9 = np.arange(129, dtype=f)[None, :]
    h8 = np.arange(128, dtype=f)[None, :]
    ehT = np.zeros((H, 258), f)
    ehT[:, 0:129] = np.cos(2 * np.pi * m * h9 / H) / 16.0
    ehT[:, 129:257] = -np.sin(2 * np.pi * m * h8 / H) / 16.0

    n = np.arange(W, dtype=f)[:, None]
    w0 = np.arange(WF, dtype=f)[None, :]
    Cw = np.cos(2 * np.pi * n * w0 / W) / 16.0
    Sw = -np.sin(2 * np.pi * n * w0 / W) / 16.0
    rhs1 = np.concatenate([Cw, Sw], 1)
    rhs2 = np.concatenate([-Sw, Cw], 1)
    rhs2n = -rhs2

    ht = np.arange(H)
    hperm = np.where(ht <= 127, ht, 383 - ht).astype(f)
    mm = np.arange(H, dtype=f)[None, :]
    Er = np.cos(2 * np.pi * hperm[:, None] * mm / H) / 16.0
    Ei = np.sin(2 * np.pi * hperm[:, None] * mm / H) / 16.0
    erei = np.concatenate([Er, Ei], 1)
    einer = np.concatenate([-Ei, Er], 1)

    w = np.arange(WF, dtype=f)[:, None]
    cw = np.where((w == 0) | (w == 128), 1.0, 2.0)
    nn_ = np.arange(W, dtype=f)[None, :]
    Amat = cw * np.cos(2 * np.pi * nn_ * w / W) / 16.0
    Bmat = -cw * np.sin(2 * np.pi * nn_ * w / W) / 16.0
    c32 = lambda a: np.ascontiguousarray(a, dtype=np.float32)
    return dict(ehT=c32(ehT), rhs1=c32(rhs1), rhs2=c32(rhs2), rhs2n=c32(rhs2n),
                erei=c32(erei), einer=c32(einer), amat=c32(Amat), bmat=c32(Bmat[0:128]))


class _Builder:
    def __init__(self):
        import concourse.bass as bass
        import concourse.bacc as bacc
        import concourse.mybir as mybir
        import concourse.tile as tile
        self.bass, self.mybir, self.tile = bass, mybir, tile
        self.dt = mybir.dt
        self.F32 = self.dt.float32
        self.BF16 = self.dt.bfloat16
        self.AF = mybir.ActivationFunctionType
        self.ALU = mybir.AluOpType
        self.nc = bacc.Bacc("TRN2")

    def declare_io(self):
        nc, F32, BF16 = self.nc, self.F32, self.BF16
        self.xin = nc.dram_tensor("xin", [UNITS_PER_CORE, CB, H, W], F32, kind="ExternalInput")
        self.xbf = nc.dram_tensor("xbf", [UNITS_PER_CORE, CB, H, W], BF16, kind="ExternalInput")
        self.wmix = nc.dram_tensor("wmix", [UNITS_PER_CORE, 6, CB + 1, CB], BF16, kind="ExternalInput")
        self.cdram = {}
        for name, shp in [("ehT", [H, 258]), ("rhs1", [H, 258]), ("rhs2", [H, 258]),
                          ("rhs2n", [H, 258]), ("erei", [H, 512]), ("einer", [H, 512]),
                          ("amat", [WF, 256]), ("bmat", [128, 256])]:
            self.cdram[name] = nc.dram_tensor("c_" + name, shp, BF16, kind="ExternalInput")
        self.out = nc.dram_tensor("out", [UNITS_PER_CORE, CB, H, W], F32, kind="ExternalOutput")
        self.zd = nc.dram_tensor("zd", [UNITS_PER_CORE, CB, H, 2, WF], BF16)
        self.sd = nc.dram_tensor("sd", [UNITS_PER_CORE, CB, H, 2, WF], BF16)

    def load_consts(self, cpool):
        nc, BF16 = self.nc, self.BF16
        t = lambda shape, tag: cpool.tile(shape, BF16, tag=tag, name=tag)
        self.ehT_sb = t([128, 2, 258], "ehT")
        nc.sync.dma_start(self.ehT_sb[:], self.cdram["ehT"][:].rearrange("(t p) f -> p t f", p=128))
        self.rhs1_sb = t([128, 2, 258], "rhs1")
        nc.sync.dma_start(self.rhs1_sb[:], self.cdram["rhs1"][:].rearrange("(t p) f -> p t f", p=128))
        self.rhs2_sb = t([128, 2, 258], "rhs2")
        nc.sync.dma_start(self.rhs2_sb[:], self.cdram["rhs2"][:].rearrange("(t p) f -> p t f", p=128))
        self.rhs2n_sb = t([128, 2, 258], "rhs2n")
        nc.sync.dma_start(self.rhs2n_sb[:], self.cdram["rhs2n"][:].rearrange("(t p) f -> p t f", p=128))
        self.erei_sb = t([128, 2, 512], "erei")
        nc.sync.dma_start(self.erei_sb[:], self.cdram["erei"][:].rearrange("(t p) f -> p t f", p=128))
        self.einer_sb = t([128, 2, 512], "einer")
        nc.sync.dma_start(self.einer_sb[:], self.cdram["einer"][:].rearrange("(t p) f -> p t f", p=128))
        self.amat_sb = t([128, 256], "amat")
        nc.sync.dma_start(self.amat_sb[:], self.cdram["amat"][0:128])
        self.anyq_sb = t([1, 256], "anyq")
        nc.sync.dma_start(self.anyq_sb[:], self.cdram["amat"][128:129])
        self.bmat_sb = t([128, 256], "bmat")
        nc.sync.dma_start(self.bmat_sb[:], self.cdram["bmat"][:])
        # persistent pass2 tiles with constant ones-rows
        self.zt_slots = [cpool.tile([CB + 1, HTG, 2, WF], BF16, tag=f"zt{i}", name=f"zt{i}") for i in range(2)]
        self.o1_slots = [cpool.tile([CB + 1, 2, 258], BF16, tag=f"o1s{i}", name=f"o1s{i}") for i in range(2)]
        for tt in self.zt_slots:
            nc.gpsimd.memset(tt[CB:CB + 1, :, :, :], 1.0)
        for tt in self.o1_slots:
            nc.gpsimd.memset(tt[CB:CB + 1, :, :], 1.0)

    def fwd_channel(self, ci, x_t, z_t, ps1t, ps1z):
        nc, F32 = self.nc, self.F32
        t1p = ps1t.tile([128, 2, 512], F32, tag="t1p", name="t1p")
        for nt in range(2):
            for mt in range(2):
                nc.tensor.matmul(t1p[:, nt, 0:258],
                                 x_t[:, ci, mt, nt * 128:(nt + 1) * 128],
                                 self.ehT_sb[:, mt, :],
                                 start=(mt == 0), stop=(mt == 1))
        t1sb = self.p1t1.tile([128, 2, 258], self.BF16, tag="t1sb", name="t1sb")
        nc.scalar.copy(t1sb[:], t1p[:, :, 0:258])
        top = ps1z.tile([128, 258], F32, tag="top", name="top")
        bot = ps1z.tile([128, 258], F32, tag="bot", name="bot")
        for kt in range(2):
            # top rows h=0..127; bot rows p -> h=255-p (p<127), p=127 -> h=128
            nc.tensor.matmul(top[:], t1sb[:, kt, 0:128], self.rhs1_sb[:, kt, :],
                             start=(kt == 0), stop=False)
            nc.tensor.matmul(bot[:], t1sb[:, kt, 1:129], self.rhs1_sb[:, kt, :],
                             start=(kt == 0), stop=False)
            nc.tensor.matmul(top[:], t1sb[:, kt, 129:257], self.rhs2_sb[:, kt, :],
                             start=False, stop=(kt == 1))
            nc.tensor.matmul(bot[:], t1sb[:, kt, 130:258], self.rhs2n_sb[:, kt, :],
                             start=False, stop=(kt == 1))
        nc.vector.tensor_copy(z_t[:, ci, 0, :], top[:])
        nc.vector.tensor_copy(z_t[:, ci, 1, :], bot[:])

    def pass1(self, j, tc):
        nc, F32 = self.nc, self.F32
        with (
            tc.tile_pool(name="ps1t", bufs=2, space="PSUM") as ps1t,
            tc.tile_pool(name="ps1z", bufs=2, space="PSUM") as ps1z,
        ):
            for cg in range(0, CB, CG):
                x_t = self.p1x.tile([128, CG, 2, 256], self.BF16, tag="x", name="x")
                nc.sync.dma_start(
                    x_t[:], self.xbf[j, cg:cg + CG].rearrange("c (t p) n -> p c t n", p=128))
                z_t = self.p1z.tile([128, CG, 2, 258], self.BF16, tag="z", name="z")
                for ci in range(CG):
                    self.fwd_channel(ci, x_t, z_t, ps1t, ps1z)
                nc.sync.dma_start(
                    self.zd[j, cg:cg + CG].rearrange("c (t p) r w -> p c t (r w)", p=128),
                    z_t[:])

    def mix_chunk(self, hh, zt, st, wsb, ps2):
        nc, F32, ALU = self.nc, self.F32, self.ALU
        o1p = ps2.tile([CB, 2, 512], F32, tag="o1p", name="o1p")
        o2p = ps2.tile([CB, 2, 512], F32, tag="o2p", name="o2p")
        zr = zt[:, hh:hh + 2, 0, :]
        zi = zt[:, hh:hh + 2, 1, :]
        o1r = o1p[:, 0, 0:258]
        o1i = o1p[:, 1, 0:258]
        nc.tensor.matmul(o1i, wsb[:, 1, :], zr, start=True, stop=False)
        nc.tensor.matmul(o1r, wsb[:, 0, :], zr, start=True, stop=False)
        nc.tensor.matmul(o1i, wsb[:, 0, :], zi, start=False, stop=True)
        nc.tensor.matmul(o1r, wsb[:, 2, :], zi, start=False, stop=True)
        o1sb = self.o1_slots[(hh // 2) % 2]
        nc.scalar.activation(o1sb[0:CB, :, :], o1p[:, :, 0:258], self.AF.Relu)
        o2r = o2p[:, 0, 0:258]
        o2i = o2p[:, 1, 0:258]
        nc.tensor.matmul(o2i, wsb[:, 4, :], o1sb[:, 0, :], start=True, stop=False)
        nc.tensor.matmul(o2r, wsb[:, 3, :], o1sb[:, 0, :], start=True, stop=False)
        nc.tensor.matmul(o2i, wsb[:, 3, :], o1sb[:, 1, :], start=False, stop=True)
        nc.tensor.matmul(o2r, wsb[:, 5, :], o1sb[:, 1, :], start=False, stop=True)
        # softshrink(u) = relu(u-lam) + (min(u,-lam)+lam)
        r1 = self.p2r.tile([CB, 2, 258], self.BF16, tag="r1", name="r1")
        r2n = self.p2r.tile([CB, 2, 258], self.BF16, tag="r2n", name="r2n")
        nc.vector.tensor_scalar(r1[:], o2p[:, :, 0:258], -LAM, 0.0, ALU.add, ALU.max)
        nc.vector.tensor_scalar(r2n[:], o2p[:, :, 0:258], -LAM, -LAM, ALU.min, ALU.subtract)
        # st free layout is [ht][ri][w]; r1/r2n are [ri][(ht,w)] — permute via APs
        st_v = st[:, hh:hh + 2, :].rearrange("c h (r w) -> c r h w", r=2)
        nc.gpsimd.tensor_add(st_v, r1[:].rearrange("c r (h w) -> c r h w", h=2),
                             r2n[:].rearrange("c r (h w) -> c r h w", h=2))

    def pass2(self, j, tc, wsb):
        nc = self.nc
        with tc.tile_pool(name="ps2", bufs=2, space="PSUM") as ps2:
            for hti in range(H // HTG):
                zt = self.zt_slots[hti % 2]
                nc.gpsimd.dma_start(zt[0:CB], self.zd[j, :, hti * HTG:(hti + 1) * HTG])
                st = self.p2s.tile([CB, HTG, 258], self.BF16, tag="st", name="st")
                for hh in range(0, HTG, 2):
                    self.mix_chunk(hh, zt, st, wsb, ps2)
                nc.sync.dma_start(
                    self.sd[j, :, hti * HTG:(hti + 1) * HTG].rearrange("c h r w -> c h (r w)"),
                    st[:])

    def inv_channel(self, ci, s4, x4, o4, ps3u, ps3n, ps3o):
        nc, F32 = self.nc, self.F32
        up = ps3u.tile([128, 512], F32, tag="up", name="up")
        unp = ps3n.tile([1, 512], F32, tag="unp", name="unp")
        for kt in range(2):
            Ap = s4[:, ci, kt, 0, 0:128]
            Bp = s4[:, ci, kt, 1, 0:128]
            nc.tensor.matmul(up[:], Ap, self.erei_sb[:, kt, :], start=(kt == 0), stop=False)
            nc.tensor.matmul(up[:], Bp, self.einer_sb[:, kt, :], start=False, stop=(kt == 1))
            nc.tensor.matmul(unp[:], s4[:, ci, kt, 0, 128:129], self.erei_sb[:, kt, :],
                             start=(kt == 0), stop=False)
            nc.tensor.matmul(unp[:], s4[:, ci, kt, 1, 128:129], self.einer_sb[:, kt, :],
                             start=False, stop=(kt == 1))
        usb = self.p3u.tile([128, 512], self.BF16, tag="usb", name="usb")
        nc.scalar.copy(usb[:], up[:])
        unsb = self.p3u.tile([1, 512], self.BF16, tag="unsb", name="unsb")
        nc.vector.tensor_copy(unsb[:], unp[:])
        for mt in range(2):
            op = ps3o.tile([128, 256], F32, tag="op", name="op")
            nc.tensor.matmul(op[:], usb[:, mt * 128:(mt + 1) * 128], self.amat_sb[:],
                             start=True, stop=False)
            nc.tensor.matmul(op[:], usb[:, 256 + mt * 128:256 + (mt + 1) * 128],
                             self.bmat_sb[:], start=False, stop=False)
            nc.tensor.matmul(op[:], unsb[:, mt * 128:(mt + 1) * 128], self.anyq_sb[:],
                             start=False, stop=True)
            nc.vector.tensor_add(o4[:, ci, mt, :], op[:], x4[:, ci, mt, :])

    def pass3(self, j, tc):
        nc, F32 = self.nc, self.F32
        with (
            tc.tile_pool(name="ps3u", bufs=2, space="PSUM") as ps3u,
            tc.tile_pool(name="ps3n", bufs=2, space="PSUM") as ps3n,
            tc.tile_pool(name="ps3o", bufs=3, space="PSUM") as ps3o,
        ):
            for cg in range(0, CB, CG):
                s4 = self.p3s.tile([128, CG, 2, 2, WF], self.BF16, tag="s4", name="s4")
                nc.sync.dma_start(
                    s4[:], self.sd[j, cg:cg + CG].rearrange("c (t p) r w -> p c t r w", p=128))
                x4 = self.p3x.tile([128, CG, 2, 256], F32, tag="x4", name="x4")
                nc.sync.dma_start(
                    x4[:], self.xin[j, cg:cg + CG].rearrange("c (t p) n -> p c t n", p=128))
                o4 = self.p3o.tile([128, CG, 2, 256], F32, tag="o4", name="o4")
                for ci in range(CG):
                    self.inv_channel(ci, s4, x4, o4, ps3u, ps3n, ps3o)
                nc.sync.dma_start(
                    self.out[j, cg:cg + CG].rearrange("c (t p) n -> p c t n", p=128), o4[:])

    def build(self):
        tile = self.tile
        self.declare_io()
        with tile.TileContext(self.nc) as tc:
            with _pools(tc, self):
                self.load_consts(self.cpool)
                for j in range(UNITS_PER_CORE):
                    wsb = self.wpool.tile([CB + 1, 6, CB], self.BF16, tag="wsb", name="wsb")
                    self.nc.sync.dma_start(wsb[:], self.wmix[j].rearrange("s k o -> k s o"))
                    self.pass1(j, tc)
                    self.pass2(j, tc, wsb)
                    self.pass3(j, tc)
        self.nc.finalize()
        return self.nc


from contextlib import contextmanager


@contextmanager
def _pools(tc, b):
    with (
        tc.tile_pool(name="consts", bufs=1) as cpool,
        tc.tile_pool(name="wpool", bufs=2) as wpool,
        tc.tile_pool(name="p1x", bufs=2) as p1x,
        tc.tile_pool(name="p1t1", bufs=3) as p1t1,
        tc.tile_pool(name="p1z", bufs=2) as p1z,
        tc.tile_pool(name="p2r", bufs=3) as p2r,
        tc.tile_pool(name="p2s", bufs=2) as p2s,
        tc.tile_pool(name="p3s", bufs=2) as p3s,
        tc.tile_pool(name="p3x", bufs=2) as p3x,
        tc.tile_pool(name="p3u", bufs=2) as p3u,
        tc.tile_pool(name="p3o", bufs=2) as p3o,
    ):
        b.cpool, b.wpool = cpool, wpool
        b.p1x, b.p1t1, b.p1z = p1x, p1t1, p1z
        b.p2r, b.p2s = p2r, p2s
        b.p3s, b.p3x, b.p3u, b.p3o = p3s, p3x, p3u, p3o
        yield None


def _build_nc():
    return _Builder().build()


def _prep_core_inputs(x, w1, b1, w2, b2, consts, cid):
    xs, ws = [], []
    for jj in range(UNITS_PER_CORE):
        p = UNITS_PER_CORE * cid + jj
        b, k = p // NB, p % NB
        xs.append(x[b, k * CB:(k + 1) * CB])
        w1r = w1[k, :, :, 0]
        w1i = w1[k, :, :, 1]
        w2r = w2[k, :, :, 0]
        w2i = w2[k, :, :, 1]
        b1r = b1[k, :, 0, 0, 0]
        b1i = b1[k, :, 0, 0, 1]
        b2r = b2[k, :, 0, 0, 0]
        b2i = b2[k, :, 0, 0, 1]
        aug = lambda mat, bias: np.concatenate(
            [mat, np.asarray(bias, np.float32)[None, :]], 0).astype(np.float32)
        zeros = np.zeros(CB, np.float32)
        ws.append(np.stack([
            aug(w1r, zeros), aug(w1i, b1i), aug(-w1i, b1r),
            aug(w2r, zeros), aug(w2i, b2i), aug(-w2i, b2r)], 0))
    import ml_dtypes
    xarr = np.ascontiguousarray(np.stack(xs, 0), dtype=np.float32)
    in_map = {
        "xin": xarr,
        "xbf": xarr.astype(ml_dtypes.bfloat16),
        "wmix": np.ascontiguousarray(np.stack(ws, 0)).astype(ml_dtypes.bfloat16),
    }
    for name in ("ehT", "rhs1", "rhs2", "rhs2n", "erei", "einer", "amat", "bmat"):
        in_map["c_" + name] = consts[name].astype(ml_dtypes.bfloat16)
    return in_map


def kernel(x, w1, b1, w2, b2):
    from concourse.bass_utils import run_bass_kernel_spmd

    x = np.asarray(x, dtype=np.float32)
    w1 = np.asarray(w1, dtype=np.float32)
    b1 = np.asarray(b1, dtype=np.float32)
    w2 = np.asarray(w2, dtype=np.float32)
    b2 = np.asarray(b2, dtype=np.float32)

    if "nc" not in _cache:
        _cache["nc"] = _build_nc()
    nc = _cache["nc"]
    consts = _make_consts()

    in_maps = [_prep_core_inputs(x, w1, b1, w2, b2, consts, cid) for cid in range(NCORES)]
    res = run_bass_kernel_spmd(nc, in_maps, core_ids=list(range(NCORES)))
    _cache["last_res"] = res

    outf = np.empty_like(x)
    for cid in range(NCORES):
        o = res.results[cid]["out"]
        for jj in range(UNITS_PER_CORE):
            p = UNITS_PER_CORE * cid + jj
            b, k = p // NB, p % NB
            outf[b, k * CB:(k + 1) * CB] = o[jj]
    return outf
